# revision 35
# baseline (speedup 1.0000x reference)
"""Multi-head attention (B=4, S=2048, D=1024, H=16, Dh=64) on 8 trn2 cores.

Sharding: core c -> batch b=c//2, head-group g=c%2 (8 heads = 512 qkv cols).
Host folds log2(e)/sqrt(Dh) into Wq/bq (so scores PSUM holds t = s*log2e),
drops bk (softmax-invariant), splits bo across the two cores of each batch.
Each core computes a transposed partial output outT [1024, 2048]; host sums
core pairs and transposes.

Per-core pipeline (single pass, engines overlapped):
  - x/weights DMA'd to SBUF up front; v projection (seq-natural, ones col per
    head for the softmax denominator), then per head-pair c: q/k projections
    (transposed, d-on-partition) interleaved into the PREVIOUS pair's
    attention slots so PE never idles.
  - attention per (pair, 512-query block): scores for both heads of the pair
    run CONCURRENTLY via PE row-tiling (K=64 each, rows 0-63 / 64-127) into
    one [128, 1024] PSUM tile = [h0 512q | h1 512q]; one exp instruction
    (ACT, scale=ln2) covers both heads. A tunable subset of key-chunks
    (DVE_KC) computes exp on the Vector engine instead via the Schraudolph
    bit-trick: int16(t*128 + B) reinterpreted as bf16 == 2^t * (1+eps),
    |eps| <= 4.2% rms 1.8%, applied to 3/16 of keys -> ~1e-3..1e-2 final.
    ctx[65, 512] += v_aug^T @ P accumulates over key chunks (row 64 = l).
  - normalize: ctx copied out of PSUM fast (frees banks), reciprocal on DVE,
    partition-broadcast on GpSimd, multiply -> cn (bf16).
  - output projection (bf16) interleaved into the last pair's slots + tail;
    bo/2 folded into the eviction; DMA out.
"""
import numpy as np
import ml_dtypes
from contextlib import ExitStack

import concourse.bass as bass
import concourse.bacc as bacc
import concourse.mybir as mybir
import concourse.tile as tile
from concourse.bass_utils import run_bass_kernel_spmd

F32 = mybir.dt.float32
F32R = mybir.dt.float32r
BF16 = mybir.dt.bfloat16
I16 = mybir.dt.int16
NP_BF16 = ml_dtypes.bfloat16

B = 4
S = 2048
D = 1024
COLS = 512          # qkv cols per core (8 heads x 64)
NHEAD = 8           # heads per core
DCH = D // 128      # 8 contraction chunks for projections
CC = 4              # 4 col chunks of 128 = 4 head pairs
KC = S // 128       # 16 key chunks of 128
QH = 4              # 4 query blocks of 512
N = 512

LOG2E = float(np.log2(np.e))
LN2 = float(np.log(2.0))
B_SCHRAUD = 16256.0 - 7.4   # bf16 Schraudolph magic bias (calibrated)
DVE_KC = (3, 7, 11, 15)   # key chunks whose exp runs on DVE (Schraudolph)

_CACHE = {}


def _build():
    nc = bacc.Bacc("TRN2", target_bir_lowering=False, debug=False, num_devices=8)

    xt = nc.declare_dram_parameter("xt", [D, S], BF16, isOutput=False)
    wqt = nc.declare_dram_parameter("wqt", [D, COLS], BF16, isOutput=False)
    wkt = nc.declare_dram_parameter("wkt", [D, COLS], BF16, isOutput=False)
    wvt = nc.declare_dram_parameter("wvt", [D, COLS], BF16, isOutput=False)
    wot = nc.declare_dram_parameter("wot", [COLS, D], BF16, isOutput=False)
    bq = nc.declare_dram_parameter("bq", [128, CC], F32, isOutput=False)
    bv = nc.declare_dram_parameter("bv", [1, COLS], F32, isOutput=False)
    bo2 = nc.declare_dram_parameter("bo2", [128, DCH], F32, isOutput=False)
    out = nc.declare_dram_parameter("out", [D, S], F32, isOutput=True)

    with ExitStack() as ctx:
        tc = ctx.enter_context(tile.TileContext(nc))

        const = ctx.enter_context(tc.tile_pool(name="const", bufs=1))
        ones_f32 = const.tile([128, 128], F32, tag="ones_f32")
        nc.vector.memset(ones_f32[:], 1.0)
        ones_r64 = const.tile([1, 64], F32, tag="ones_r64")
        nc.vector.memset(ones_r64[:], 1.0)
        # preload the exp table set early (one tiny activation)
        warm = const.tile([128, 8], BF16, tag="warm")
        nc.scalar.activation(warm[:], ones_f32[:, 0:8],
                             mybir.ActivationFunctionType.Exp)

        # ---------------- resident inputs ----------------
        # DMA priority: wq/wk + x quarter 0 feed the qk0 lead; wv + later x
        # quarters feed the v projection embedded in pair-0 qh0; wo last.
        wsb = ctx.enter_context(tc.tile_pool(name="wsb", bufs=1))
        xs = [[None] * QH for _ in range(DCH)]
        wq_all = wsb.tile([128, DCH * COLS], BF16, tag="wqa", name="wq_all")
        nc.sync.dma_start(
            out=wq_all[:].rearrange("p (d c) -> p d c", c=COLS),
            in_=wqt[:].rearrange("(d p) c -> p d c", p=128))
        wq_sb = [wq_all[:, d * COLS:(d + 1) * COLS] for d in range(DCH)]
        x0_all = wsb.tile([128, DCH * N], BF16, tag="xa0", name="x_all0")
        nc.sync.dma_start(
            out=x0_all[:].rearrange("p (d c) -> p d c", c=N),
            in_=xt[:, 0:N].rearrange("(d p) c -> p d c", p=128))
        for d in range(DCH):
            xs[d][0] = x0_all[:, d * N:(d + 1) * N]
        wk_all = wsb.tile([128, DCH * COLS], BF16, tag="wka", name="wk_all")
        nc.sync.dma_start(
            out=wk_all[:].rearrange("p (d c) -> p d c", c=COLS),
            in_=wkt[:].rearrange("(d p) c -> p d c", p=128))
        wk_sb = [wk_all[:, d * COLS:(d + 1) * COLS] for d in range(DCH)]
        bq_t = const.tile([128, CC], F32, tag="bq")
        nc.sync.dma_start(out=bq_t[:], in_=bq[:])
        bv_t = const.tile([1, COLS], F32, tag="bv")
        nc.sync.dma_start(out=bv_t[:], in_=bv[:])
        bvb = const.tile([128, COLS], F32, tag="bvb")
        nc.gpsimd.partition_broadcast(bvb[:], bv_t[:])
        bo_t = const.tile([128, DCH], F32, tag="bo")
        nc.sync.dma_start(out=bo_t[:], in_=bo2[:])
        # bulk inputs as single wide DMAs (d-chunks side by side) to cut
        # per-dma issue overhead
        wv_all = wsb.tile([128, DCH * COLS], BF16, tag="wva", name="wv_all")
        nc.sync.dma_start(
            out=wv_all[:].rearrange("p (d c) -> p d c", c=COLS),
            in_=wvt[:].rearrange("(d p) c -> p d c", p=128))
        wv_sb = [wv_all[:, d * COLS:(d + 1) * COLS] for d in range(DCH)]
        for q in range(1, QH):
            xq = wsb.tile([128, DCH * N], BF16, tag=f"xa{q}", name=f"x_all{q}")
            nc.sync.dma_start(
                out=xq[:].rearrange("p (d c) -> p d c", c=N),
                in_=xt[:, q * N:(q + 1) * N].rearrange("(d p) c -> p d c", p=128))
            for d in range(DCH):
                xs[d][q] = xq[:, d * N:(d + 1) * N]
        wo_all = wsb.tile([128, CC * D], BF16, tag="woa", name="wo_all")
        nc.sync.dma_start(
            out=wo_all[:].rearrange("p (c e) -> p c e", e=D),
            in_=wot[:].rearrange("(c p) e -> p c e", p=128))
        wo_sb = [wo_all[:, c2 * D:(c2 + 1) * D] for c2 in range(CC)]

        # ---------------- persistent activations ----------------
        qkv = ctx.enter_context(tc.tile_pool(name="qkv", bufs=1))
        qT = [qkv.tile([128, S], BF16, tag=f"qt{c}", name=f"qt{c}") for c in range(CC)]
        kT = [qkv.tile([128, S], BF16, tag=f"kt{c}", name=f"kt{c}") for c in range(CC)]
        v_sb = [qkv.tile([128, NHEAD * 65], BF16, tag=f"v{i}", name=f"v{i}")
                for i in range(KC)]
        cn = [qkv.tile([128, S], BF16, tag=f"cn{c}", name=f"cn{c}") for c in range(CC)]

        for i in range(KC):
            va = v_sb[i][:].rearrange("p (h c) -> p h c", c=65)
            nc.vector.tensor_copy(
                out=va[:, :, 64:65],
                in_=ones_f32[:, 0:NHEAD].rearrange("p (h c) -> p h c", c=1),
            )

        # ---------------- work pools ----------------
        pp = ctx.enter_context(tc.tile_pool(name="pp", bufs=2, space="PSUM"))
        stp = ctx.enter_context(tc.tile_pool(name="stp", bufs=2, space="PSUM"))
        cxp = ctx.enter_context(tc.tile_pool(name="cxp", bufs=2, space="PSUM"))
        pb = ctx.enter_context(tc.tile_pool(name="pb", bufs=4))
        ip = ctx.enter_context(tc.tile_pool(name="ip", bufs=4))
        crp = ctx.enter_context(tc.tile_pool(name="crp", bufs=6))
        rp = ctx.enter_context(tc.tile_pool(name="rp", bufs=2))
        osb = ctx.enter_context(tc.tile_pool(name="osb", bufs=2))

        def emit_v_chunk(s16):
            h, off = s16 // 4, (s16 % 4) * 128
            vps = pp.tile([128, N], F32, tag="pp", name=f"vps{s16}")
            for d in range(DCH):
                nc.tensor.matmul(
                    vps[:], xs[d][h][:, off:off + 128], wv_sb[d][:],
                    start=(d == 0), stop=(d == DCH - 1))
            dst = v_sb[s16][:].rearrange("p (h c) -> p h c", c=65)
            src = vps[:].rearrange("p (h c) -> p h c", c=64)
            nc.vector.tensor_tensor(
                out=dst[:, :, 0:64], in0=src[:],
                in1=bvb[:].rearrange("p (h c) -> p h c", c=64),
                op=mybir.AluOpType.add)

        def emit_qk_tile(proj, c, sc):
            wsrc = wq_sb if proj == "q" else wk_sb
            dst = qT if proj == "q" else kT
            ps = pp.tile([128, N], F32, tag="pp", name=f"{proj}ps{c}_{sc}")
            for d in range(DCH):
                nc.tensor.matmul(
                    ps[:], wsrc[d][:, c * 128:(c + 1) * 128],
                    xs[d][sc][:],
                    start=(d == 0), stop=(d == DCH - 1))
            if proj == "q":
                nc.vector.tensor_scalar_add(
                    out=dst[c][:, sc * N:(sc + 1) * N], in0=ps[:],
                    scalar1=bq_t[:, c:c + 1])
            else:
                nc.vector.tensor_copy(
                    out=dst[c][:, sc * N:(sc + 1) * N], in_=ps[:])

        def emit_ph3_tile(e, qc):
            ps = pp.tile([128, N], F32, tag="pp", name=f"ops{e}_{qc}")
            for c2 in range(CC):
                nc.tensor.matmul(
                    ps[:], wo_sb[c2][:, e * 128:(e + 1) * 128],
                    cn[c2][:, qc * N:(qc + 1) * N],
                    start=(c2 == 0), stop=(c2 == CC - 1))
            o_t = osb.tile([128, N], F32, tag="osb", name=f"osb{e}_{qc}")
            nc.vector.tensor_scalar_add(out=o_t[:], in0=ps[:],
                                        scalar1=bo_t[:, e:e + 1])
            nc.sync.dma_start(
                out=out[e * 128:(e + 1) * 128, qc * N:(qc + 1) * N], in_=o_t[:])

        # ---------------- lead-in: qk for pair 0 (quarter-ordered) ----------------
        for sc in range(QH):
            emit_qk_tile("q", 0, sc)
            emit_qk_tile("k", 0, sc)

        # ---------------- attention (+ interleaved proj / out-proj) ----------------
        for c in range(CC):
            # extra PE work to interleave into this pair's iteration slots:
            # pair 0 qh0 hosts the v projection (1 chunk per kc iteration);
            # pairs 0-2 host the next pair's q/k projection; pair 3 hosts the
            # first 3 query-blocks of the output projection.
            extras = {qh: [] for qh in range(QH)}
            if c == 0:
                i = 0
                for proj in ("q", "k"):
                    for sc in range(QH):
                        extras[1 + i * 3 // 8].append(("qk", (proj, 1, sc)))
                        i += 1
            elif c < 3:
                i = 0
                for proj in ("q", "k"):
                    for sc in range(QH):
                        extras[i // 2].append(("qk", (proj, c + 1, sc)))
                        i += 1
            else:
                for qc in range(3):
                    for e in range(DCH):
                        extras[min(qc + 1, 3)].append(("ph3", (e, qc)))

            for qh in range(QH):
                q0 = qh * N
                ctx0 = cxp.tile([65, N], F32, tag="ctx", name=f"ctx0_{c}_{qh}")
                ctx1 = cxp.tile([65, N], F32, tag="ctx", name=f"ctx1_{c}_{qh}")
                slot = list(extras[qh])
                si = 0

                def emit_ctx(kc, pap):
                    nc.tensor.matmul(
                        ctx0[:], v_sb[kc][:, (2 * c) * 65:(2 * c) * 65 + 65],
                        pap[:, 0:N], start=(kc == 0), stop=(kc == KC - 1))
                    nc.tensor.matmul(
                        ctx1[:], v_sb[kc][:, (2 * c + 1) * 65:(2 * c + 1) * 65 + 65],
                        pap[:, N:2 * N], start=(kc == 0), stop=(kc == KC - 1))

                pend = []  # software-pipeline: ctx trails scores/exp by two kc
                for kc in range(KC):
                    st = stp.tile([128, 1024], F32, tag="st", name=f"st{c}_{qh}_{kc}")
                    # scores for both heads, concurrent via PE row tiling
                    nc.tensor.matmul(
                        st[:, 0:N],
                        kT[c][0:64, kc * 128:(kc + 1) * 128],
                        qT[c][0:64, q0:q0 + N], start=True, stop=True)
                    nc.tensor.matmul(
                        st[:, N:2 * N],
                        kT[c][64:128, kc * 128:(kc + 1) * 128],
                        qT[c][64:128, q0:q0 + N], start=True, stop=True)
                    if kc in DVE_KC:
                        it = ip.tile([128, 1024], I16, tag="ip", name=f"it{c}_{qh}_{kc}")
                        nc.vector.tensor_scalar(
                            out=it[:], in0=st[:],
                            scalar1=128.0, scalar2=B_SCHRAUD,
                            op0=mybir.AluOpType.mult, op1=mybir.AluOpType.add)
                        pap = it.bitcast(BF16)
                    else:
                        p_t = pb.tile([128, 1024], BF16, tag="pb", name=f"p{c}_{qh}_{kc}")
                        nc.scalar.activation(
                            p_t[:], st[:], mybir.ActivationFunctionType.Exp,
                            scale=LN2)
                        pap = p_t
                    if c == 0 and qh == 0:
                        emit_v_chunk(kc)
                    pend.append((kc, pap))
                    if len(pend) > 3:
                        emit_ctx(*pend.pop(0))
                    if kc % 4 == 3 and si < len(slot):
                        budget = 2 if c == 3 else 1
                        for _ in range(budget):
                            if si >= len(slot):
                                break
                            kind, args = slot[si]
                            si += 1
                            if kind == "qk":
                                emit_qk_tile(*args)
                            else:
                                emit_ph3_tile(*args)
                for it_ in pend:
                    emit_ctx(*it_)
                # leftover extras (shouldn't happen, but keep correct)
                while si < len(slot):
                    kind, args = slot[si]
                    si += 1
                    if kind == "qk":
                        emit_qk_tile(*args)
                    else:
                        emit_ph3_tile(*args)
                # normalize both heads. Only the raw PSUM->SBUF copies are on
                # the ctx-buffer critical path; the multiplies are emitted
                # last so the DVE FIFO never stalls on the gpsimd broadcast.
                crs = []
                for hh, cps in ((0, ctx0), (1, ctx1)):
                    cr = crp.tile([65, N], F32, tag="crp", name=f"cr{c}_{qh}_{hh}")
                    nc.scalar.activation(cr[:], cps[:],
                                         mybir.ActivationFunctionType.Copy)
                    crs.append(cr)
                rbs = []
                for hh, cr in enumerate(crs):
                    l_t = rp.tile([1, N], F32, tag="lp", name=f"l{c}_{qh}_{hh}")
                    nc.vector.tensor_copy(out=l_t[:], in_=cr[64:65, :])
                    r_t = rp.tile([1, N], F32, tag="rp", name=f"r{c}_{qh}_{hh}")
                    nc.vector.reciprocal_approx_fast(r_t[:], l_t[:])
                    rb_f = pp.tile([128, N], F32, tag="pp", name=f"rb{c}_{qh}_{hh}")
                    nc.tensor.matmul(rb_f[0:64, :], ones_r64[:], r_t[:],
                                     start=True, stop=True)
                    rbs.append(rb_f[0:64, :])
                for hh, (cr, rb_t) in enumerate(zip(crs, rbs)):
                    nc.vector.tensor_tensor(
                        out=cn[c][hh * 64:hh * 64 + 64, q0:q0 + N],
                        in0=cr[0:64, :], in1=rb_t[:],
                        op=mybir.AluOpType.mult)

        # ---------------- out-projection tail (qc=3) ----------------
        for e in range(DCH):
            emit_ph3_tile(e, 3)

    nc.compile()
    return nc


def _get_nc():
    if "nc" not in _CACHE:
        _CACHE["nc"] = _build()
    return _CACHE["nc"]


def _in_maps(x, Wq, bq, Wk, Wv, bv, Wo, bo):
    qs = LOG2E / 8.0
    maps = []
    for core in range(8):
        b, g = core // 2, core % 2
        cols = slice(g * COLS, (g + 1) * COLS)
        maps.append({
            "xt": np.ascontiguousarray(x[b].T).astype(NP_BF16),
            "wqt": np.ascontiguousarray((Wq[cols] * qs).T).astype(NP_BF16),
            "bq": np.ascontiguousarray((bq[cols] * qs).reshape(CC, 128).T),
            "wkt": np.ascontiguousarray(Wk[cols].T).astype(NP_BF16),
            "wvt": np.ascontiguousarray(Wv[cols].T).astype(NP_BF16),
            "bv": bv[cols].reshape(1, COLS).copy(),
            "wot": np.ascontiguousarray(Wo[:, cols].T).astype(NP_BF16),
            "bo2": np.ascontiguousarray((bo / 2.0).reshape(DCH, 128).T),
        })
    return maps


def kernel(x, Wq, bq, Wk, bk, Wv, bv, Wo, bo, _trace=False, **trace_kwargs):
    x = np.asarray(x, dtype=np.float32)
    Wq = np.asarray(Wq, dtype=np.float32)
    bq = np.asarray(bq, dtype=np.float32)
    Wk = np.asarray(Wk, dtype=np.float32)
    Wv = np.asarray(Wv, dtype=np.float32)
    bv = np.asarray(bv, dtype=np.float32)
    Wo = np.asarray(Wo, dtype=np.float32)
    bo = np.asarray(bo, dtype=np.float32)

    nc = _get_nc()
    maps = _in_maps(x, Wq, bq, Wk, Wv, bv, Wo, bo)
    res = run_bass_kernel_spmd(nc, maps, list(range(8)), trace=_trace, **trace_kwargs)

    outp = np.empty((B, S, D), np.float32)
    for b in range(B):
        t = res.results[2 * b]["out"] + res.results[2 * b + 1]["out"]
        outp[b] = t.T
    if _trace:
        return outp, res
    return outp


# revision 36
# speedup vs baseline: 1.2089x; 1.2089x over previous
"""Multi-head attention (B=4, S=2048, D=1024, H=16, Dh=64) on 8 trn2 cores.

Sharding: core c -> batch b=c//2, head-group g=c%2 (8 heads = 512 qkv cols).
Host folds log2(e)/sqrt(Dh) into Wq/bq (so scores PSUM holds t = s*log2e),
drops bk (softmax-invariant), splits bo across the two cores of each batch.
Each core computes a transposed partial output outT [1024, 2048]; host sums
core pairs and transposes.

Per-core pipeline (single pass, engines overlapped):
  - x/weights DMA'd to SBUF up front; v projection (seq-natural, ones col per
    head for the softmax denominator), then per head-pair c: q/k projections
    (transposed, d-on-partition) interleaved into the PREVIOUS pair's
    attention slots so PE never idles.
  - attention per (pair, 512-query block): scores for both heads of the pair
    run CONCURRENTLY via PE row-tiling (K=64 each, rows 0-63 / 64-127) into
    one [128, 1024] PSUM tile = [h0 512q | h1 512q]; one exp instruction
    (ACT, scale=ln2) covers both heads. A tunable subset of key-chunks
    (DVE_KC) computes exp on the Vector engine instead via the Schraudolph
    bit-trick: int16(t*128 + B) reinterpreted as bf16 == 2^t * (1+eps),
    |eps| <= 4.2% rms 1.8%, applied to 3/16 of keys -> ~1e-3..1e-2 final.
    ctx[65, 512] += v_aug^T @ P accumulates over key chunks (row 64 = l).
  - normalize: ctx copied out of PSUM fast (frees banks), reciprocal on DVE,
    partition-broadcast on GpSimd, multiply -> cn (bf16).
  - output projection (bf16) interleaved into the last pair's slots + tail;
    bo/2 folded into the eviction; DMA out.
"""
import numpy as np
import ml_dtypes
from contextlib import ExitStack

import concourse.bass as bass
import concourse.bacc as bacc
import concourse.mybir as mybir
import concourse.tile as tile
from concourse.bass_utils import run_bass_kernel_spmd

F32 = mybir.dt.float32
F32R = mybir.dt.float32r
BF16 = mybir.dt.bfloat16
I16 = mybir.dt.int16
NP_BF16 = ml_dtypes.bfloat16

B = 4
S = 2048
D = 1024
COLS = 512          # qkv cols per core (8 heads x 64)
NHEAD = 8           # heads per core
DCH = D // 128      # 8 contraction chunks for projections
CC = 4              # 4 col chunks of 128 = 4 head pairs
KC = S // 128       # 16 key chunks of 128
QH = 4              # 4 query blocks of 512
N = 512

LOG2E = float(np.log2(np.e))
LN2 = float(np.log(2.0))
B_SCHRAUD = 16256.0 - 7.4   # bf16 Schraudolph magic bias (calibrated)
DVE_KC = (3, 7, 11, 15)   # key chunks whose exp runs on DVE (Schraudolph)

_CACHE = {}


def _build():
    nc = bacc.Bacc("TRN2", target_bir_lowering=False, debug=False, num_devices=8)

    xt = nc.declare_dram_parameter("xt", [D, S], BF16, isOutput=False)
    wqt = nc.declare_dram_parameter("wqt", [D, COLS], BF16, isOutput=False)
    wkt = nc.declare_dram_parameter("wkt", [D, COLS], BF16, isOutput=False)
    wvt = nc.declare_dram_parameter("wvt", [D, COLS], BF16, isOutput=False)
    wot = nc.declare_dram_parameter("wot", [COLS, D], BF16, isOutput=False)
    bq = nc.declare_dram_parameter("bq", [128, CC], F32, isOutput=False)
    bv = nc.declare_dram_parameter("bv", [1, COLS], F32, isOutput=False)
    bo2 = nc.declare_dram_parameter("bo2", [128, DCH], F32, isOutput=False)
    out = nc.declare_dram_parameter("out", [D, S], F32, isOutput=True)

    with ExitStack() as ctx:
        tc = ctx.enter_context(tile.TileContext(nc))

        const = ctx.enter_context(tc.tile_pool(name="const", bufs=1))
        ones_f32 = const.tile([128, 128], F32, tag="ones_f32")
        nc.vector.memset(ones_f32[:], 1.0)
        # preload the exp table set early (one tiny activation)
        warm = const.tile([128, 8], BF16, tag="warm")
        nc.scalar.activation(warm[:], ones_f32[:, 0:8],
                             mybir.ActivationFunctionType.Exp)

        # ---------------- resident inputs ----------------
        # DMA priority: wq/wk + x quarter 0 feed the qk0 lead; wv + later x
        # quarters feed the v projection embedded in pair-0 qh0; wo last.
        wsb = ctx.enter_context(tc.tile_pool(name="wsb", bufs=1))
        xs = [[None] * QH for _ in range(DCH)]
        wq_all = wsb.tile([128, DCH * COLS], BF16, tag="wqa", name="wq_all")
        nc.sync.dma_start(
            out=wq_all[:].rearrange("p (d c) -> p d c", c=COLS),
            in_=wqt[:].rearrange("(d p) c -> p d c", p=128))
        wq_sb = [wq_all[:, d * COLS:(d + 1) * COLS] for d in range(DCH)]
        x0_all = wsb.tile([128, DCH * N], BF16, tag="xa0", name="x_all0")
        nc.sync.dma_start(
            out=x0_all[:].rearrange("p (d c) -> p d c", c=N),
            in_=xt[:, 0:N].rearrange("(d p) c -> p d c", p=128))
        for d in range(DCH):
            xs[d][0] = x0_all[:, d * N:(d + 1) * N]
        wk_all = wsb.tile([128, DCH * COLS], BF16, tag="wka", name="wk_all")
        nc.sync.dma_start(
            out=wk_all[:].rearrange("p (d c) -> p d c", c=COLS),
            in_=wkt[:].rearrange("(d p) c -> p d c", p=128))
        wk_sb = [wk_all[:, d * COLS:(d + 1) * COLS] for d in range(DCH)]
        bq_t = const.tile([128, CC], F32, tag="bq")
        nc.sync.dma_start(out=bq_t[:], in_=bq[:])
        bv_t = const.tile([1, COLS], F32, tag="bv")
        nc.sync.dma_start(out=bv_t[:], in_=bv[:])
        bvb = const.tile([128, COLS], F32, tag="bvb")
        nc.gpsimd.partition_broadcast(bvb[:], bv_t[:])
        bo_t = const.tile([128, DCH], F32, tag="bo")
        nc.sync.dma_start(out=bo_t[:], in_=bo2[:])
        # bulk inputs as single wide DMAs (d-chunks side by side) to cut
        # per-dma issue overhead
        wv_all = wsb.tile([128, DCH * COLS], BF16, tag="wva", name="wv_all")
        nc.sync.dma_start(
            out=wv_all[:].rearrange("p (d c) -> p d c", c=COLS),
            in_=wvt[:].rearrange("(d p) c -> p d c", p=128))
        wv_sb = [wv_all[:, d * COLS:(d + 1) * COLS] for d in range(DCH)]
        for q in range(1, QH):
            xq = wsb.tile([128, DCH * N], BF16, tag=f"xa{q}", name=f"x_all{q}")
            nc.sync.dma_start(
                out=xq[:].rearrange("p (d c) -> p d c", c=N),
                in_=xt[:, q * N:(q + 1) * N].rearrange("(d p) c -> p d c", p=128))
            for d in range(DCH):
                xs[d][q] = xq[:, d * N:(d + 1) * N]
        wo_all = wsb.tile([128, CC * D], BF16, tag="woa", name="wo_all")
        nc.sync.dma_start(
            out=wo_all[:].rearrange("p (c e) -> p c e", e=D),
            in_=wot[:].rearrange("(c p) e -> p c e", p=128))
        wo_sb = [wo_all[:, c2 * D:(c2 + 1) * D] for c2 in range(CC)]

        # ---------------- persistent activations ----------------
        qkv = ctx.enter_context(tc.tile_pool(name="qkv", bufs=1))
        qT = [qkv.tile([128, S], BF16, tag=f"qt{c}", name=f"qt{c}") for c in range(CC)]
        kT = [qkv.tile([128, S], BF16, tag=f"kt{c}", name=f"kt{c}") for c in range(CC)]
        v_sb = [qkv.tile([128, NHEAD * 65], BF16, tag=f"v{i}", name=f"v{i}")
                for i in range(KC)]
        cn = [qkv.tile([128, S], BF16, tag=f"cn{c}", name=f"cn{c}") for c in range(CC)]

        for i in range(KC):
            va = v_sb[i][:].rearrange("p (h c) -> p h c", c=65)
            nc.vector.tensor_copy(
                out=va[:, :, 64:65],
                in_=ones_f32[:, 0:NHEAD].rearrange("p (h c) -> p h c", c=1),
            )

        # ---------------- work pools ----------------
        pp = ctx.enter_context(tc.tile_pool(name="pp", bufs=2, space="PSUM"))
        stp = ctx.enter_context(tc.tile_pool(name="stp", bufs=2, space="PSUM"))
        cxp = ctx.enter_context(tc.tile_pool(name="cxp", bufs=2, space="PSUM"))
        pb = ctx.enter_context(tc.tile_pool(name="pb", bufs=4))
        ip = ctx.enter_context(tc.tile_pool(name="ip", bufs=4))
        crp = ctx.enter_context(tc.tile_pool(name="crp", bufs=6))
        rp = ctx.enter_context(tc.tile_pool(name="rp", bufs=2))
        rbp = ctx.enter_context(tc.tile_pool(name="rbp", bufs=2))
        osb = ctx.enter_context(tc.tile_pool(name="osb", bufs=2))

        def emit_v_chunk(s16):
            h, off = s16 // 4, (s16 % 4) * 128
            vps = pp.tile([128, N], F32, tag="pp", name=f"vps{s16}")
            for d in range(DCH):
                nc.tensor.matmul(
                    vps[:], xs[d][h][:, off:off + 128], wv_sb[d][:],
                    start=(d == 0), stop=(d == DCH - 1))
            dst = v_sb[s16][:].rearrange("p (h c) -> p h c", c=65)
            src = vps[:].rearrange("p (h c) -> p h c", c=64)
            nc.vector.tensor_tensor(
                out=dst[:, :, 0:64], in0=src[:],
                in1=bvb[:].rearrange("p (h c) -> p h c", c=64),
                op=mybir.AluOpType.add)

        def emit_qk_tile(proj, c, sc):
            wsrc = wq_sb if proj == "q" else wk_sb
            dst = qT if proj == "q" else kT
            ps = pp.tile([128, N], F32, tag="pp", name=f"{proj}ps{c}_{sc}")
            for d in range(DCH):
                nc.tensor.matmul(
                    ps[:], wsrc[d][:, c * 128:(c + 1) * 128],
                    xs[d][sc][:],
                    start=(d == 0), stop=(d == DCH - 1))
            if proj == "q":
                nc.vector.tensor_scalar_add(
                    out=dst[c][:, sc * N:(sc + 1) * N], in0=ps[:],
                    scalar1=bq_t[:, c:c + 1])
            else:
                nc.vector.tensor_copy(
                    out=dst[c][:, sc * N:(sc + 1) * N], in_=ps[:])

        def emit_ph3_tile(e, qc):
            ps = pp.tile([128, N], F32, tag="pp", name=f"ops{e}_{qc}")
            for c2 in range(CC):
                nc.tensor.matmul(
                    ps[:], wo_sb[c2][:, e * 128:(e + 1) * 128],
                    cn[c2][:, qc * N:(qc + 1) * N],
                    start=(c2 == 0), stop=(c2 == CC - 1))
            o_t = osb.tile([128, N], F32, tag="osb", name=f"osb{e}_{qc}")
            nc.vector.tensor_scalar_add(out=o_t[:], in0=ps[:],
                                        scalar1=bo_t[:, e:e + 1])
            nc.sync.dma_start(
                out=out[e * 128:(e + 1) * 128, qc * N:(qc + 1) * N], in_=o_t[:])

        # ---------------- lead-in: qk for pair 0 (quarter-ordered) ----------------
        for sc in range(QH):
            emit_qk_tile("q", 0, sc)
            emit_qk_tile("k", 0, sc)

        # ---------------- attention (+ interleaved proj / out-proj) ----------------
        for c in range(CC):
            # extra PE work to interleave into this pair's iteration slots:
            # pair 0 qh0 hosts the v projection (1 chunk per kc iteration);
            # pairs 0-2 host the next pair's q/k projection; pair 3 hosts the
            # first 3 query-blocks of the output projection.
            extras = {qh: [] for qh in range(QH)}
            if c == 0:
                i = 0
                for proj in ("q", "k"):
                    for sc in range(QH):
                        extras[1 + i * 3 // 8].append(("qk", (proj, 1, sc)))
                        i += 1
            elif c < 3:
                i = 0
                for proj in ("q", "k"):
                    for sc in range(QH):
                        extras[i // 2].append(("qk", (proj, c + 1, sc)))
                        i += 1
            else:
                for qc in range(3):
                    for e in range(DCH):
                        extras[min(qc + 1, 3)].append(("ph3", (e, qc)))

            for qh in range(QH):
                q0 = qh * N
                ctx0 = cxp.tile([65, N], F32, tag="ctx", name=f"ctx0_{c}_{qh}")
                ctx1 = cxp.tile([65, N], F32, tag="ctx", name=f"ctx1_{c}_{qh}")
                slot = list(extras[qh])
                si = 0

                def emit_ctx(kc, pap):
                    nc.tensor.matmul(
                        ctx0[:], v_sb[kc][:, (2 * c) * 65:(2 * c) * 65 + 65],
                        pap[:, 0:N], start=(kc == 0), stop=(kc == KC - 1))
                    nc.tensor.matmul(
                        ctx1[:], v_sb[kc][:, (2 * c + 1) * 65:(2 * c + 1) * 65 + 65],
                        pap[:, N:2 * N], start=(kc == 0), stop=(kc == KC - 1))

                pend = []  # software-pipeline: ctx trails scores/exp by two kc
                for kc in range(KC):
                    st = stp.tile([128, 1024], F32, tag="st", name=f"st{c}_{qh}_{kc}")
                    # scores for both heads, concurrent via PE row tiling
                    nc.tensor.matmul(
                        st[:, 0:N],
                        kT[c][0:64, kc * 128:(kc + 1) * 128],
                        qT[c][0:64, q0:q0 + N], start=True, stop=True)
                    nc.tensor.matmul(
                        st[:, N:2 * N],
                        kT[c][64:128, kc * 128:(kc + 1) * 128],
                        qT[c][64:128, q0:q0 + N], start=True, stop=True)
                    if kc in DVE_KC:
                        it = ip.tile([128, 1024], I16, tag="ip", name=f"it{c}_{qh}_{kc}")
                        nc.vector.tensor_scalar(
                            out=it[:], in0=st[:],
                            scalar1=128.0, scalar2=B_SCHRAUD,
                            op0=mybir.AluOpType.mult, op1=mybir.AluOpType.add)
                        pap = it.bitcast(BF16)
                    else:
                        p_t = pb.tile([128, 1024], BF16, tag="pb", name=f"p{c}_{qh}_{kc}")
                        nc.scalar.activation(
                            p_t[:], st[:], mybir.ActivationFunctionType.Exp,
                            scale=LN2)
                        pap = p_t
                    if c == 0 and qh == 0:
                        emit_v_chunk(kc)
                    pend.append((kc, pap))
                    if len(pend) > 3:
                        emit_ctx(*pend.pop(0))
                    if kc % 4 == 3 and si < len(slot):
                        budget = 2 if c == 3 else 1
                        for _ in range(budget):
                            if si >= len(slot):
                                break
                            kind, args = slot[si]
                            si += 1
                            if kind == "qk":
                                emit_qk_tile(*args)
                            else:
                                emit_ph3_tile(*args)
                for it_ in pend:
                    emit_ctx(*it_)
                # leftover extras (shouldn't happen, but keep correct)
                while si < len(slot):
                    kind, args = slot[si]
                    si += 1
                    if kind == "qk":
                        emit_qk_tile(*args)
                    else:
                        emit_ph3_tile(*args)
                # normalize both heads. Only the raw PSUM->SBUF copies are on
                # the ctx-buffer critical path; the multiplies are emitted
                # last so the DVE FIFO never stalls on the gpsimd broadcast.
                crs = []
                for hh, cps in ((0, ctx0), (1, ctx1)):
                    cr = crp.tile([65, N], F32, tag="crp", name=f"cr{c}_{qh}_{hh}")
                    nc.scalar.activation(cr[:], cps[:],
                                         mybir.ActivationFunctionType.Copy)
                    crs.append(cr)
                rbs = []
                for hh, cr in enumerate(crs):
                    l_t = rp.tile([1, N], F32, tag="lp", name=f"l{c}_{qh}_{hh}")
                    nc.vector.tensor_copy(out=l_t[:], in_=cr[64:65, :])
                    r_t = rp.tile([1, N], F32, tag="rp", name=f"r{c}_{qh}_{hh}")
                    nc.vector.reciprocal_approx_fast(r_t[:], l_t[:])
                    rb_t = rbp.tile([64, N], F32, tag="rbp", name=f"rb{c}_{qh}_{hh}")
                    nc.gpsimd.partition_broadcast(rb_t[:], r_t[:])
                    rbs.append(rb_t)
                for hh, (cr, rb_t) in enumerate(zip(crs, rbs)):
                    nc.vector.tensor_tensor(
                        out=cn[c][hh * 64:hh * 64 + 64, q0:q0 + N],
                        in0=cr[0:64, :], in1=rb_t[:],
                        op=mybir.AluOpType.mult)

        # ---------------- out-projection tail (qc=3) ----------------
        for e in range(DCH):
            emit_ph3_tile(e, 3)

    nc.compile()
    return nc


def _get_nc():
    if "nc" not in _CACHE:
        _CACHE["nc"] = _build()
    return _CACHE["nc"]


def _in_maps(x, Wq, bq, Wk, Wv, bv, Wo, bo):
    qs = LOG2E / 8.0
    maps = []
    for core in range(8):
        b, g = core // 2, core % 2
        cols = slice(g * COLS, (g + 1) * COLS)
        maps.append({
            "xt": np.ascontiguousarray(x[b].T).astype(NP_BF16),
            "wqt": np.ascontiguousarray((Wq[cols] * qs).T).astype(NP_BF16),
            "bq": np.ascontiguousarray((bq[cols] * qs).reshape(CC, 128).T),
            "wkt": np.ascontiguousarray(Wk[cols].T).astype(NP_BF16),
            "wvt": np.ascontiguousarray(Wv[cols].T).astype(NP_BF16),
            "bv": bv[cols].reshape(1, COLS).copy(),
            "wot": np.ascontiguousarray(Wo[:, cols].T).astype(NP_BF16),
            "bo2": np.ascontiguousarray((bo / 2.0).reshape(DCH, 128).T),
        })
    return maps


def kernel(x, Wq, bq, Wk, bk, Wv, bv, Wo, bo, _trace=False, **trace_kwargs):
    x = np.asarray(x, dtype=np.float32)
    Wq = np.asarray(Wq, dtype=np.float32)
    bq = np.asarray(bq, dtype=np.float32)
    Wk = np.asarray(Wk, dtype=np.float32)
    Wv = np.asarray(Wv, dtype=np.float32)
    bv = np.asarray(bv, dtype=np.float32)
    Wo = np.asarray(Wo, dtype=np.float32)
    bo = np.asarray(bo, dtype=np.float32)

    nc = _get_nc()
    maps = _in_maps(x, Wq, bq, Wk, Wv, bv, Wo, bo)
    res = run_bass_kernel_spmd(nc, maps, list(range(8)), trace=_trace, **trace_kwargs)

    outp = np.empty((B, S, D), np.float32)
    for b in range(B):
        t = res.results[2 * b]["out"] + res.results[2 * b + 1]["out"]
        outp[b] = t.T
    if _trace:
        return outp, res
    return outp


# revision 37
# speedup vs baseline: 1.2290x; 1.0166x over previous
"""Multi-head attention (B=4, S=2048, D=1024, H=16, Dh=64) on 8 trn2 cores.

Sharding: core c -> batch b=c//2, head-group g=c%2 (8 heads = 512 qkv cols).
Host folds log2(e)/sqrt(Dh) into Wq/bq (so scores PSUM holds t = s*log2e),
drops bk (softmax-invariant), splits bo across the two cores of each batch.
Each core computes a transposed partial output outT [1024, 2048]; host sums
core pairs and transposes.

Per-core pipeline (single pass, engines overlapped):
  - x/weights DMA'd to SBUF up front; v projection (seq-natural, ones col per
    head for the softmax denominator), then per head-pair c: q/k projections
    (transposed, d-on-partition) interleaved into the PREVIOUS pair's
    attention slots so PE never idles.
  - attention per (pair, 512-query block): scores for both heads of the pair
    run CONCURRENTLY via PE row-tiling (K=64 each, rows 0-63 / 64-127) into
    one [128, 1024] PSUM tile = [h0 512q | h1 512q]; one exp instruction
    (ACT, scale=ln2) covers both heads. A tunable subset of key-chunks
    (DVE_KC) computes exp on the Vector engine instead via the Schraudolph
    bit-trick: int16(t*128 + B) reinterpreted as bf16 == 2^t * (1+eps),
    |eps| <= 4.2% rms 1.8%, applied to 3/16 of keys -> ~1e-3..1e-2 final.
    ctx[65, 512] += v_aug^T @ P accumulates over key chunks (row 64 = l).
  - normalize: ctx copied out of PSUM fast (frees banks), reciprocal on DVE,
    partition-broadcast on GpSimd, multiply -> cn (bf16).
  - output projection (bf16) interleaved into the last pair's slots + tail;
    bo/2 folded into the eviction; DMA out.
"""
import numpy as np
import ml_dtypes
from contextlib import ExitStack

import concourse.bass as bass
import concourse.bacc as bacc
import concourse.mybir as mybir
import concourse.tile as tile
from concourse.bass_utils import run_bass_kernel_spmd

F32 = mybir.dt.float32
F32R = mybir.dt.float32r
BF16 = mybir.dt.bfloat16
I16 = mybir.dt.int16
NP_BF16 = ml_dtypes.bfloat16

B = 4
S = 2048
D = 1024
COLS = 512          # qkv cols per core (8 heads x 64)
NHEAD = 8           # heads per core
DCH = D // 128      # 8 contraction chunks for projections
CC = 4              # 4 col chunks of 128 = 4 head pairs
KC = S // 128       # 16 key chunks of 128
QH = 4              # 4 query blocks of 512
N = 512

LOG2E = float(np.log2(np.e))
LN2 = float(np.log(2.0))
B_SCHRAUD = 16256.0 - 7.4   # bf16 Schraudolph magic bias (calibrated)
DVE_KC = (2, 6, 10, 14)   # key chunks whose exp runs on DVE (Schraudolph)

_CACHE = {}


def _build():
    nc = bacc.Bacc("TRN2", target_bir_lowering=False, debug=False, num_devices=8)

    xt = nc.declare_dram_parameter("xt", [D, S], BF16, isOutput=False)
    wqt = nc.declare_dram_parameter("wqt", [D, COLS], BF16, isOutput=False)
    wkt = nc.declare_dram_parameter("wkt", [D, COLS], BF16, isOutput=False)
    wvt = nc.declare_dram_parameter("wvt", [D, COLS], BF16, isOutput=False)
    wot = nc.declare_dram_parameter("wot", [COLS, D], BF16, isOutput=False)
    bq = nc.declare_dram_parameter("bq", [128, CC], F32, isOutput=False)
    bv = nc.declare_dram_parameter("bv", [1, COLS], F32, isOutput=False)
    bo2 = nc.declare_dram_parameter("bo2", [128, DCH], F32, isOutput=False)
    out = nc.declare_dram_parameter("out", [D, S], F32, isOutput=True)

    with ExitStack() as ctx:
        tc = ctx.enter_context(tile.TileContext(nc))

        const = ctx.enter_context(tc.tile_pool(name="const", bufs=1))
        ones_f32 = const.tile([128, 128], F32, tag="ones_f32")
        nc.vector.memset(ones_f32[:], 1.0)
        # preload the exp table set early (one tiny activation)
        warm = const.tile([128, 8], BF16, tag="warm")
        nc.scalar.activation(warm[:], ones_f32[:, 0:8],
                             mybir.ActivationFunctionType.Exp)

        # ---------------- resident inputs ----------------
        # DMA priority: wq/wk + x quarter 0 feed the qk0 lead; wv + later x
        # quarters feed the v projection embedded in pair-0 qh0; wo last.
        wsb = ctx.enter_context(tc.tile_pool(name="wsb", bufs=1))
        xs = [[None] * QH for _ in range(DCH)]
        wq_all = wsb.tile([128, DCH * COLS], BF16, tag="wqa", name="wq_all")
        nc.sync.dma_start(
            out=wq_all[:].rearrange("p (d c) -> p d c", c=COLS),
            in_=wqt[:].rearrange("(d p) c -> p d c", p=128))
        wq_sb = [wq_all[:, d * COLS:(d + 1) * COLS] for d in range(DCH)]
        x0_all = wsb.tile([128, DCH * N], BF16, tag="xa0", name="x_all0")
        nc.sync.dma_start(
            out=x0_all[:].rearrange("p (d c) -> p d c", c=N),
            in_=xt[:, 0:N].rearrange("(d p) c -> p d c", p=128))
        for d in range(DCH):
            xs[d][0] = x0_all[:, d * N:(d + 1) * N]
        wk_all = wsb.tile([128, DCH * COLS], BF16, tag="wka", name="wk_all")
        nc.sync.dma_start(
            out=wk_all[:].rearrange("p (d c) -> p d c", c=COLS),
            in_=wkt[:].rearrange("(d p) c -> p d c", p=128))
        wk_sb = [wk_all[:, d * COLS:(d + 1) * COLS] for d in range(DCH)]
        bq_t = const.tile([128, CC], F32, tag="bq")
        nc.sync.dma_start(out=bq_t[:], in_=bq[:])
        bv_t = const.tile([1, COLS], F32, tag="bv")
        nc.sync.dma_start(out=bv_t[:], in_=bv[:])
        bvb = const.tile([128, COLS], F32, tag="bvb")
        nc.gpsimd.partition_broadcast(bvb[:], bv_t[:])
        bo_t = const.tile([128, DCH], F32, tag="bo")
        nc.sync.dma_start(out=bo_t[:], in_=bo2[:])
        # bulk inputs as single wide DMAs (d-chunks side by side) to cut
        # per-dma issue overhead
        wv_all = wsb.tile([128, DCH * COLS], BF16, tag="wva", name="wv_all")
        nc.sync.dma_start(
            out=wv_all[:].rearrange("p (d c) -> p d c", c=COLS),
            in_=wvt[:].rearrange("(d p) c -> p d c", p=128))
        wv_sb = [wv_all[:, d * COLS:(d + 1) * COLS] for d in range(DCH)]
        for q in range(1, QH):
            xq = wsb.tile([128, DCH * N], BF16, tag=f"xa{q}", name=f"x_all{q}")
            nc.sync.dma_start(
                out=xq[:].rearrange("p (d c) -> p d c", c=N),
                in_=xt[:, q * N:(q + 1) * N].rearrange("(d p) c -> p d c", p=128))
            for d in range(DCH):
                xs[d][q] = xq[:, d * N:(d + 1) * N]
        wo_all = wsb.tile([128, CC * D], BF16, tag="woa", name="wo_all")
        nc.sync.dma_start(
            out=wo_all[:].rearrange("p (c e) -> p c e", e=D),
            in_=wot[:].rearrange("(c p) e -> p c e", p=128))
        wo_sb = [wo_all[:, c2 * D:(c2 + 1) * D] for c2 in range(CC)]

        # ---------------- persistent activations ----------------
        qkv = ctx.enter_context(tc.tile_pool(name="qkv", bufs=1))
        qT = [qkv.tile([128, S], BF16, tag=f"qt{c}", name=f"qt{c}") for c in range(CC)]
        kT = [qkv.tile([128, S], BF16, tag=f"kt{c}", name=f"kt{c}") for c in range(CC)]
        v_sb = [qkv.tile([128, NHEAD * 65], BF16, tag=f"v{i}", name=f"v{i}")
                for i in range(KC)]
        cn = [qkv.tile([128, S], BF16, tag=f"cn{c}", name=f"cn{c}") for c in range(CC)]

        for i in range(KC):
            va = v_sb[i][:].rearrange("p (h c) -> p h c", c=65)
            nc.vector.tensor_copy(
                out=va[:, :, 64:65],
                in_=ones_f32[:, 0:NHEAD].rearrange("p (h c) -> p h c", c=1),
            )

        # ---------------- work pools ----------------
        pp = ctx.enter_context(tc.tile_pool(name="pp", bufs=2, space="PSUM"))
        stp = ctx.enter_context(tc.tile_pool(name="stp", bufs=2, space="PSUM"))
        cxp = ctx.enter_context(tc.tile_pool(name="cxp", bufs=2, space="PSUM"))
        pb = ctx.enter_context(tc.tile_pool(name="pb", bufs=4))
        ip = ctx.enter_context(tc.tile_pool(name="ip", bufs=4))
        crp = ctx.enter_context(tc.tile_pool(name="crp", bufs=8))
        rp = ctx.enter_context(tc.tile_pool(name="rp", bufs=4))
        rbp = ctx.enter_context(tc.tile_pool(name="rbp", bufs=2))
        osb = ctx.enter_context(tc.tile_pool(name="osb", bufs=6))

        def emit_v_chunk(s16):
            h, off = s16 // 4, (s16 % 4) * 128
            vps = pp.tile([128, N], F32, tag="pp", name=f"vps{s16}")
            for d in range(DCH):
                nc.tensor.matmul(
                    vps[:], xs[d][h][:, off:off + 128], wv_sb[d][:],
                    start=(d == 0), stop=(d == DCH - 1))
            dst = v_sb[s16][:].rearrange("p (h c) -> p h c", c=65)
            src = vps[:].rearrange("p (h c) -> p h c", c=64)
            nc.vector.tensor_tensor(
                out=dst[:, :, 0:64], in0=src[:],
                in1=bvb[:].rearrange("p (h c) -> p h c", c=64),
                op=mybir.AluOpType.add)

        def emit_qk_tile(proj, c, sc):
            wsrc = wq_sb if proj == "q" else wk_sb
            dst = qT if proj == "q" else kT
            ps = pp.tile([128, N], F32, tag="pp", name=f"{proj}ps{c}_{sc}")
            for d in range(DCH):
                nc.tensor.matmul(
                    ps[:], wsrc[d][:, c * 128:(c + 1) * 128],
                    xs[d][sc][:],
                    start=(d == 0), stop=(d == DCH - 1))
            if proj == "q":
                nc.vector.tensor_scalar_add(
                    out=dst[c][:, sc * N:(sc + 1) * N], in0=ps[:],
                    scalar1=bq_t[:, c:c + 1])
            else:
                nc.vector.tensor_copy(
                    out=dst[c][:, sc * N:(sc + 1) * N], in_=ps[:])

        def emit_ph3_tile(e, qc):
            ps = pp.tile([128, N], F32, tag="pp", name=f"ops{e}_{qc}")
            for c2 in range(CC):
                nc.tensor.matmul(
                    ps[:], wo_sb[c2][:, e * 128:(e + 1) * 128],
                    cn[c2][:, qc * N:(qc + 1) * N],
                    start=(c2 == 0), stop=(c2 == CC - 1))
            o_t = osb.tile([128, N], F32, tag="osb", name=f"osb{e}_{qc}")
            nc.vector.tensor_scalar_add(out=o_t[:], in0=ps[:],
                                        scalar1=bo_t[:, e:e + 1])
            nc.sync.dma_start(
                out=out[e * 128:(e + 1) * 128, qc * N:(qc + 1) * N], in_=o_t[:])

        # ---------------- lead-in: qk for pair 0 (quarter-ordered) ----------------
        for sc in range(QH):
            emit_qk_tile("q", 0, sc)
            emit_qk_tile("k", 0, sc)

        # ---------------- attention (+ interleaved proj / out-proj) ----------------
        for c in range(CC):
            # extra PE work to interleave into this pair's iteration slots:
            # pair 0 qh0 hosts the v projection (1 chunk per kc iteration);
            # pairs 0-2 host the next pair's q/k projection; pair 3 hosts the
            # first 3 query-blocks of the output projection.
            extras = {qh: [] for qh in range(QH)}
            if c == 0:
                i = 0
                for proj in ("q", "k"):
                    for sc in range(QH):
                        extras[1 + i * 3 // 8].append(("qk", (proj, 1, sc)))
                        i += 1
            elif c < 3:
                i = 0
                for proj in ("q", "k"):
                    for sc in range(QH):
                        extras[i // 2].append(("qk", (proj, c + 1, sc)))
                        i += 1
            else:
                for qc in range(3):
                    for e in range(DCH):
                        extras[min(qc + 1, 3)].append(("ph3", (e, qc)))

            for qh in range(QH):
                q0 = qh * N
                ctx0 = cxp.tile([65, N], F32, tag="ctx", name=f"ctx0_{c}_{qh}")
                ctx1 = cxp.tile([65, N], F32, tag="ctx", name=f"ctx1_{c}_{qh}")
                slot = list(extras[qh])
                si = 0

                def emit_ctx(kc, pap):
                    nc.tensor.matmul(
                        ctx0[:], v_sb[kc][:, (2 * c) * 65:(2 * c) * 65 + 65],
                        pap[:, 0:N], start=(kc == 0), stop=(kc == KC - 1))
                    nc.tensor.matmul(
                        ctx1[:], v_sb[kc][:, (2 * c + 1) * 65:(2 * c + 1) * 65 + 65],
                        pap[:, N:2 * N], start=(kc == 0), stop=(kc == KC - 1))

                pend = []  # software-pipeline: ctx trails scores/exp by two kc
                for kc in range(KC):
                    st = stp.tile([128, 1024], F32, tag="st", name=f"st{c}_{qh}_{kc}")
                    # scores for both heads, concurrent via PE row tiling
                    nc.tensor.matmul(
                        st[:, 0:N],
                        kT[c][0:64, kc * 128:(kc + 1) * 128],
                        qT[c][0:64, q0:q0 + N], start=True, stop=True)
                    nc.tensor.matmul(
                        st[:, N:2 * N],
                        kT[c][64:128, kc * 128:(kc + 1) * 128],
                        qT[c][64:128, q0:q0 + N], start=True, stop=True)
                    if kc in DVE_KC:
                        it = ip.tile([128, 1024], I16, tag="ip", name=f"it{c}_{qh}_{kc}")
                        nc.vector.tensor_scalar(
                            out=it[:], in0=st[:],
                            scalar1=128.0, scalar2=B_SCHRAUD,
                            op0=mybir.AluOpType.mult, op1=mybir.AluOpType.add)
                        pap = it.bitcast(BF16)
                    else:
                        p_t = pb.tile([128, 1024], BF16, tag="pb", name=f"p{c}_{qh}_{kc}")
                        nc.scalar.activation(
                            p_t[:], st[:], mybir.ActivationFunctionType.Exp,
                            scale=LN2)
                        pap = p_t
                    if c == 0 and qh == 0:
                        emit_v_chunk(kc)
                    pend.append((kc, pap))
                    if len(pend) > 3:
                        emit_ctx(*pend.pop(0))
                    if kc % 4 == 3 and si < len(slot):
                        budget = 2 if c == 3 else 1
                        for _ in range(budget):
                            if si >= len(slot):
                                break
                            kind, args = slot[si]
                            si += 1
                            if kind == "qk":
                                emit_qk_tile(*args)
                            else:
                                emit_ph3_tile(*args)
                for it_ in pend:
                    emit_ctx(*it_)
                # leftover extras (shouldn't happen, but keep correct)
                while si < len(slot):
                    kind, args = slot[si]
                    si += 1
                    if kind == "qk":
                        emit_qk_tile(*args)
                    else:
                        emit_ph3_tile(*args)
                # normalize both heads. Only the raw PSUM->SBUF copies are on
                # the ctx-buffer critical path; the multiplies are emitted
                # last so the DVE FIFO never stalls on the gpsimd broadcast.
                crs = []
                for hh, cps in ((0, ctx0), (1, ctx1)):
                    cr = crp.tile([65, N], F32, tag="crp", name=f"cr{c}_{qh}_{hh}")
                    nc.scalar.activation(cr[:], cps[:],
                                         mybir.ActivationFunctionType.Copy)
                    crs.append(cr)
                rbs = []
                for hh, cr in enumerate(crs):
                    l_t = rp.tile([1, N], F32, tag="lp", name=f"l{c}_{qh}_{hh}")
                    nc.vector.tensor_copy(out=l_t[:], in_=cr[64:65, :])
                    r_t = rp.tile([1, N], F32, tag="rp", name=f"r{c}_{qh}_{hh}")
                    nc.vector.reciprocal_approx_fast(r_t[:], l_t[:])
                    rb_t = rbp.tile([64, N], F32, tag="rbp", name=f"rb{c}_{qh}_{hh}")
                    nc.gpsimd.partition_broadcast(rb_t[:], r_t[:])
                    rbs.append(rb_t)
                for hh, (cr, rb_t) in enumerate(zip(crs, rbs)):
                    nc.vector.tensor_tensor(
                        out=cn[c][hh * 64:hh * 64 + 64, q0:q0 + N],
                        in0=cr[0:64, :], in1=rb_t[:],
                        op=mybir.AluOpType.mult)

        # ---------------- out-projection tail (qc=3) ----------------
        for e in range(DCH):
            emit_ph3_tile(e, 3)

    nc.compile()
    return nc


def _get_nc():
    if "nc" not in _CACHE:
        _CACHE["nc"] = _build()
    return _CACHE["nc"]


def _in_maps(x, Wq, bq, Wk, Wv, bv, Wo, bo):
    qs = LOG2E / 8.0
    maps = []
    for core in range(8):
        b, g = core // 2, core % 2
        cols = slice(g * COLS, (g + 1) * COLS)
        maps.append({
            "xt": np.ascontiguousarray(x[b].T).astype(NP_BF16),
            "wqt": np.ascontiguousarray((Wq[cols] * qs).T).astype(NP_BF16),
            "bq": np.ascontiguousarray((bq[cols] * qs).reshape(CC, 128).T),
            "wkt": np.ascontiguousarray(Wk[cols].T).astype(NP_BF16),
            "wvt": np.ascontiguousarray(Wv[cols].T).astype(NP_BF16),
            "bv": bv[cols].reshape(1, COLS).copy(),
            "wot": np.ascontiguousarray(Wo[:, cols].T).astype(NP_BF16),
            "bo2": np.ascontiguousarray((bo / 2.0).reshape(DCH, 128).T),
        })
    return maps


def kernel(x, Wq, bq, Wk, bk, Wv, bv, Wo, bo, _trace=False, **trace_kwargs):
    x = np.asarray(x, dtype=np.float32)
    Wq = np.asarray(Wq, dtype=np.float32)
    bq = np.asarray(bq, dtype=np.float32)
    Wk = np.asarray(Wk, dtype=np.float32)
    Wv = np.asarray(Wv, dtype=np.float32)
    bv = np.asarray(bv, dtype=np.float32)
    Wo = np.asarray(Wo, dtype=np.float32)
    bo = np.asarray(bo, dtype=np.float32)

    nc = _get_nc()
    maps = _in_maps(x, Wq, bq, Wk, Wv, bv, Wo, bo)
    res = run_bass_kernel_spmd(nc, maps, list(range(8)), trace=_trace, **trace_kwargs)

    outp = np.empty((B, S, D), np.float32)
    for b in range(B):
        t = res.results[2 * b]["out"] + res.results[2 * b + 1]["out"]
        outp[b] = t.T
    if _trace:
        return outp, res
    return outp


# revision 38
# speedup vs baseline: 1.2388x; 1.0079x over previous
"""Multi-head attention (B=4, S=2048, D=1024, H=16, Dh=64) on 8 trn2 cores.

Sharding: core c -> batch b=c//2, head-group g=c%2 (8 heads = 512 qkv cols).
Host folds log2(e)/sqrt(Dh) into Wq/bq (so scores PSUM holds t = s*log2e),
drops bk (softmax-invariant), splits bo across the two cores of each batch.
Each core computes a transposed partial output outT [1024, 2048]; host sums
core pairs and transposes.

Per-core pipeline (single pass, engines overlapped):
  - x/weights DMA'd to SBUF up front; v projection (seq-natural, ones col per
    head for the softmax denominator), then per head-pair c: q/k projections
    (transposed, d-on-partition) interleaved into the PREVIOUS pair's
    attention slots so PE never idles.
  - attention per (pair, 512-query block): scores for both heads of the pair
    run CONCURRENTLY via PE row-tiling (K=64 each, rows 0-63 / 64-127) into
    one [128, 1024] PSUM tile = [h0 512q | h1 512q]; one exp instruction
    (ACT, scale=ln2) covers both heads. A tunable subset of key-chunks
    (DVE_KC) computes exp on the Vector engine instead via the Schraudolph
    bit-trick: int16(t*128 + B) reinterpreted as bf16 == 2^t * (1+eps),
    |eps| <= 4.2% rms 1.8%, applied to 3/16 of keys -> ~1e-3..1e-2 final.
    ctx[65, 512] += v_aug^T @ P accumulates over key chunks (row 64 = l).
  - normalize: ctx copied out of PSUM fast (frees banks), reciprocal on DVE,
    partition-broadcast on GpSimd, multiply -> cn (bf16).
  - output projection (bf16) interleaved into the last pair's slots + tail;
    bo/2 folded into the eviction; DMA out.
"""
import numpy as np
import ml_dtypes
from contextlib import ExitStack

import concourse.bass as bass
import concourse.bacc as bacc
import concourse.mybir as mybir
import concourse.tile as tile
from concourse.bass_utils import run_bass_kernel_spmd

F32 = mybir.dt.float32
F32R = mybir.dt.float32r
BF16 = mybir.dt.bfloat16
I16 = mybir.dt.int16
NP_BF16 = ml_dtypes.bfloat16

B = 4
S = 2048
D = 1024
COLS = 512          # qkv cols per core (8 heads x 64)
NHEAD = 8           # heads per core
DCH = D // 128      # 8 contraction chunks for projections
CC = 4              # 4 col chunks of 128 = 4 head pairs
KC = S // 128       # 16 key chunks of 128
QH = 4              # 4 query blocks of 512
N = 512

LOG2E = float(np.log2(np.e))
LN2 = float(np.log(2.0))
B_SCHRAUD = 16256.0 - 7.4   # bf16 Schraudolph magic bias (calibrated)
DVE_KC = (2, 6, 10, 14)   # key chunks whose exp runs on DVE (Schraudolph)

_CACHE = {}


def _build():
    nc = bacc.Bacc("TRN2", target_bir_lowering=False, debug=False, num_devices=8)

    xt = nc.declare_dram_parameter("xt", [D, S], BF16, isOutput=False)
    wqt = nc.declare_dram_parameter("wqt", [D, COLS], BF16, isOutput=False)
    wkt = nc.declare_dram_parameter("wkt", [D, COLS], BF16, isOutput=False)
    wvt = nc.declare_dram_parameter("wvt", [D, COLS], BF16, isOutput=False)
    wot = nc.declare_dram_parameter("wot", [COLS, D], BF16, isOutput=False)
    bq = nc.declare_dram_parameter("bq", [128, CC], F32, isOutput=False)
    bv = nc.declare_dram_parameter("bv", [1, COLS], F32, isOutput=False)
    bo2 = nc.declare_dram_parameter("bo2", [128, DCH], F32, isOutput=False)
    out = nc.declare_dram_parameter("out", [D, S], F32, isOutput=True)

    with ExitStack() as ctx:
        tc = ctx.enter_context(tile.TileContext(nc))

        const = ctx.enter_context(tc.tile_pool(name="const", bufs=1))
        ones_f32 = const.tile([128, 128], F32, tag="ones_f32")
        nc.vector.memset(ones_f32[:], 1.0)
        # preload the exp table set early (one tiny activation)
        warm = const.tile([128, 8], BF16, tag="warm")
        nc.scalar.activation(warm[:], ones_f32[:, 0:8],
                             mybir.ActivationFunctionType.Exp)

        # ---------------- resident inputs ----------------
        # DMA priority: wq/wk + x quarter 0 feed the qk0 lead; wv + later x
        # quarters feed the v projection embedded in pair-0 qh0; wo last.
        wsb = ctx.enter_context(tc.tile_pool(name="wsb", bufs=1))
        xs = [[None] * QH for _ in range(DCH)]
        wq_all = wsb.tile([128, DCH * COLS], BF16, tag="wqa", name="wq_all")
        nc.sync.dma_start(
            out=wq_all[:].rearrange("p (d c) -> p d c", c=COLS),
            in_=wqt[:].rearrange("(d p) c -> p d c", p=128))
        wq_sb = [wq_all[:, d * COLS:(d + 1) * COLS] for d in range(DCH)]
        x0_all = wsb.tile([128, DCH * N], BF16, tag="xa0", name="x_all0")
        nc.sync.dma_start(
            out=x0_all[:].rearrange("p (d c) -> p d c", c=N),
            in_=xt[:, 0:N].rearrange("(d p) c -> p d c", p=128))
        for d in range(DCH):
            xs[d][0] = x0_all[:, d * N:(d + 1) * N]
        wk_all = wsb.tile([128, DCH * COLS], BF16, tag="wka", name="wk_all")
        nc.sync.dma_start(
            out=wk_all[:].rearrange("p (d c) -> p d c", c=COLS),
            in_=wkt[:].rearrange("(d p) c -> p d c", p=128))
        wk_sb = [wk_all[:, d * COLS:(d + 1) * COLS] for d in range(DCH)]
        bq_t = const.tile([128, CC], F32, tag="bq")
        nc.sync.dma_start(out=bq_t[:], in_=bq[:])
        bv_t = const.tile([1, COLS], F32, tag="bv")
        nc.sync.dma_start(out=bv_t[:], in_=bv[:])
        bvb = const.tile([128, COLS], F32, tag="bvb")
        nc.gpsimd.partition_broadcast(bvb[:], bv_t[:])
        bo_t = const.tile([128, DCH], F32, tag="bo")
        nc.sync.dma_start(out=bo_t[:], in_=bo2[:])
        # bulk inputs as single wide DMAs (d-chunks side by side) to cut
        # per-dma issue overhead
        wv_all = wsb.tile([128, DCH * COLS], BF16, tag="wva", name="wv_all")
        nc.sync.dma_start(
            out=wv_all[:].rearrange("p (d c) -> p d c", c=COLS),
            in_=wvt[:].rearrange("(d p) c -> p d c", p=128))
        wv_sb = [wv_all[:, d * COLS:(d + 1) * COLS] for d in range(DCH)]
        for q in range(1, QH):
            xq = wsb.tile([128, DCH * N], BF16, tag=f"xa{q}", name=f"x_all{q}")
            nc.sync.dma_start(
                out=xq[:].rearrange("p (d c) -> p d c", c=N),
                in_=xt[:, q * N:(q + 1) * N].rearrange("(d p) c -> p d c", p=128))
            for d in range(DCH):
                xs[d][q] = xq[:, d * N:(d + 1) * N]
        wo_all = wsb.tile([128, CC * D], BF16, tag="woa", name="wo_all")
        nc.sync.dma_start(
            out=wo_all[:].rearrange("p (c e) -> p c e", e=D),
            in_=wot[:].rearrange("(c p) e -> p c e", p=128))
        wo_sb = [wo_all[:, c2 * D:(c2 + 1) * D] for c2 in range(CC)]

        # ---------------- persistent activations ----------------
        qkv = ctx.enter_context(tc.tile_pool(name="qkv", bufs=1))
        qT = [qkv.tile([128, S], BF16, tag=f"qt{c}", name=f"qt{c}") for c in range(CC)]
        kT = [qkv.tile([128, S], BF16, tag=f"kt{c}", name=f"kt{c}") for c in range(CC)]
        v_sb = [qkv.tile([128, NHEAD * 65], BF16, tag=f"v{i}", name=f"v{i}")
                for i in range(KC)]
        cn = [qkv.tile([128, S], BF16, tag=f"cn{c}", name=f"cn{c}") for c in range(CC)]

        for i in range(KC):
            va = v_sb[i][:].rearrange("p (h c) -> p h c", c=65)
            nc.vector.tensor_copy(
                out=va[:, :, 64:65],
                in_=ones_f32[:, 0:NHEAD].rearrange("p (h c) -> p h c", c=1),
            )

        # ---------------- work pools ----------------
        pp = ctx.enter_context(tc.tile_pool(name="pp", bufs=2, space="PSUM"))
        stp = ctx.enter_context(tc.tile_pool(name="stp", bufs=2, space="PSUM"))
        cxp = ctx.enter_context(tc.tile_pool(name="cxp", bufs=2, space="PSUM"))
        pb = ctx.enter_context(tc.tile_pool(name="pb", bufs=5))
        ip = ctx.enter_context(tc.tile_pool(name="ip", bufs=5))
        crp = ctx.enter_context(tc.tile_pool(name="crp", bufs=8))
        rp = ctx.enter_context(tc.tile_pool(name="rp", bufs=4))
        rbp = ctx.enter_context(tc.tile_pool(name="rbp", bufs=2))
        osb = ctx.enter_context(tc.tile_pool(name="osb", bufs=6))

        def emit_v_chunk(s16):
            h, off = s16 // 4, (s16 % 4) * 128
            vps = pp.tile([128, N], F32, tag="pp", name=f"vps{s16}")
            for d in range(DCH):
                nc.tensor.matmul(
                    vps[:], xs[d][h][:, off:off + 128], wv_sb[d][:],
                    start=(d == 0), stop=(d == DCH - 1))
            dst = v_sb[s16][:].rearrange("p (h c) -> p h c", c=65)
            src = vps[:].rearrange("p (h c) -> p h c", c=64)
            nc.vector.tensor_tensor(
                out=dst[:, :, 0:64], in0=src[:],
                in1=bvb[:].rearrange("p (h c) -> p h c", c=64),
                op=mybir.AluOpType.add)

        def emit_qk_tile(proj, c, sc):
            wsrc = wq_sb if proj == "q" else wk_sb
            dst = qT if proj == "q" else kT
            ps = pp.tile([128, N], F32, tag="pp", name=f"{proj}ps{c}_{sc}")
            for d in range(DCH):
                nc.tensor.matmul(
                    ps[:], wsrc[d][:, c * 128:(c + 1) * 128],
                    xs[d][sc][:],
                    start=(d == 0), stop=(d == DCH - 1))
            if proj == "q":
                nc.vector.tensor_scalar_add(
                    out=dst[c][:, sc * N:(sc + 1) * N], in0=ps[:],
                    scalar1=bq_t[:, c:c + 1])
            else:
                nc.vector.tensor_copy(
                    out=dst[c][:, sc * N:(sc + 1) * N], in_=ps[:])

        def emit_ph3_tile(e, qc):
            ps = pp.tile([128, N], F32, tag="pp", name=f"ops{e}_{qc}")
            for c2 in range(CC):
                nc.tensor.matmul(
                    ps[:], wo_sb[c2][:, e * 128:(e + 1) * 128],
                    cn[c2][:, qc * N:(qc + 1) * N],
                    start=(c2 == 0), stop=(c2 == CC - 1))
            o_t = osb.tile([128, N], F32, tag="osb", name=f"osb{e}_{qc}")
            nc.vector.tensor_scalar_add(out=o_t[:], in0=ps[:],
                                        scalar1=bo_t[:, e:e + 1])
            nc.sync.dma_start(
                out=out[e * 128:(e + 1) * 128, qc * N:(qc + 1) * N], in_=o_t[:])

        # ---------------- lead-in: qk for pair 0 (quarter-ordered) ----------------
        for sc in range(QH):
            emit_qk_tile("q", 0, sc)
            emit_qk_tile("k", 0, sc)

        # ---------------- attention (+ interleaved proj / out-proj) ----------------
        for c in range(CC):
            # extra PE work to interleave into this pair's iteration slots:
            # pair 0 qh0 hosts the v projection (1 chunk per kc iteration);
            # pairs 0-2 host the next pair's q/k projection; pair 3 hosts the
            # first 3 query-blocks of the output projection.
            extras = {qh: [] for qh in range(QH)}
            if c == 0:
                i = 0
                for proj in ("q", "k"):
                    for sc in range(QH):
                        extras[1 + i * 3 // 8].append(("qk", (proj, 1, sc)))
                        i += 1
            elif c < 3:
                i = 0
                for proj in ("q", "k"):
                    for sc in range(QH):
                        extras[i // 2].append(("qk", (proj, c + 1, sc)))
                        i += 1
            else:
                for qc in range(3):
                    for e in range(DCH):
                        extras[min(qc + 1, 3)].append(("ph3", (e, qc)))

            for qh in range(QH):
                q0 = qh * N
                ctx0 = cxp.tile([65, N], F32, tag="ctx", name=f"ctx0_{c}_{qh}")
                ctx1 = cxp.tile([65, N], F32, tag="ctx", name=f"ctx1_{c}_{qh}")
                slot = list(extras[qh])
                si = 0

                def emit_ctx(kc, pap):
                    nc.tensor.matmul(
                        ctx0[:], v_sb[kc][:, (2 * c) * 65:(2 * c) * 65 + 65],
                        pap[:, 0:N], start=(kc == 0), stop=(kc == KC - 1))
                    nc.tensor.matmul(
                        ctx1[:], v_sb[kc][:, (2 * c + 1) * 65:(2 * c + 1) * 65 + 65],
                        pap[:, N:2 * N], start=(kc == 0), stop=(kc == KC - 1))

                pend = []  # software-pipeline: ctx trails scores/exp by two kc
                for kc in range(KC):
                    st = stp.tile([128, 1024], F32, tag="st", name=f"st{c}_{qh}_{kc}")
                    # scores for both heads, concurrent via PE row tiling
                    nc.tensor.matmul(
                        st[:, 0:N],
                        kT[c][0:64, kc * 128:(kc + 1) * 128],
                        qT[c][0:64, q0:q0 + N], start=True, stop=True)
                    nc.tensor.matmul(
                        st[:, N:2 * N],
                        kT[c][64:128, kc * 128:(kc + 1) * 128],
                        qT[c][64:128, q0:q0 + N], start=True, stop=True)
                    if kc in DVE_KC:
                        it = ip.tile([128, 1024], I16, tag="ip", name=f"it{c}_{qh}_{kc}")
                        nc.vector.tensor_scalar(
                            out=it[:], in0=st[:],
                            scalar1=128.0, scalar2=B_SCHRAUD,
                            op0=mybir.AluOpType.mult, op1=mybir.AluOpType.add)
                        pap = it.bitcast(BF16)
                    else:
                        p_t = pb.tile([128, 1024], BF16, tag="pb", name=f"p{c}_{qh}_{kc}")
                        nc.scalar.activation(
                            p_t[:], st[:], mybir.ActivationFunctionType.Exp,
                            scale=LN2)
                        pap = p_t
                    if c == 0 and qh == 0:
                        emit_v_chunk(kc)
                    pend.append((kc, pap))
                    if len(pend) > 3:
                        emit_ctx(*pend.pop(0))
                    if kc % 4 == 3 and si < len(slot):
                        budget = 2 if c == 3 else 1
                        for _ in range(budget):
                            if si >= len(slot):
                                break
                            kind, args = slot[si]
                            si += 1
                            if kind == "qk":
                                emit_qk_tile(*args)
                            else:
                                emit_ph3_tile(*args)
                for it_ in pend:
                    emit_ctx(*it_)
                # leftover extras (shouldn't happen, but keep correct)
                while si < len(slot):
                    kind, args = slot[si]
                    si += 1
                    if kind == "qk":
                        emit_qk_tile(*args)
                    else:
                        emit_ph3_tile(*args)
                # normalize both heads. Only the raw PSUM->SBUF copies are on
                # the ctx-buffer critical path; the multiplies are emitted
                # last so the DVE FIFO never stalls on the gpsimd broadcast.
                crs = []
                for hh, cps in ((0, ctx0), (1, ctx1)):
                    cr = crp.tile([65, N], F32, tag="crp", name=f"cr{c}_{qh}_{hh}")
                    nc.scalar.activation(cr[:], cps[:],
                                         mybir.ActivationFunctionType.Copy)
                    crs.append(cr)
                rbs = []
                for hh, cr in enumerate(crs):
                    l_t = rp.tile([1, N], F32, tag="lp", name=f"l{c}_{qh}_{hh}")
                    nc.vector.tensor_copy(out=l_t[:], in_=cr[64:65, :])
                    r_t = rp.tile([1, N], F32, tag="rp", name=f"r{c}_{qh}_{hh}")
                    nc.vector.reciprocal_approx_fast(r_t[:], l_t[:])
                    rb_t = rbp.tile([64, N], F32, tag="rbp", name=f"rb{c}_{qh}_{hh}")
                    nc.gpsimd.partition_broadcast(rb_t[:], r_t[:])
                    rbs.append(rb_t)
                for hh, (cr, rb_t) in enumerate(zip(crs, rbs)):
                    nc.vector.tensor_tensor(
                        out=cn[c][hh * 64:hh * 64 + 64, q0:q0 + N],
                        in0=cr[0:64, :], in1=rb_t[:],
                        op=mybir.AluOpType.mult)

        # ---------------- out-projection tail (qc=3) ----------------
        for e in range(DCH):
            emit_ph3_tile(e, 3)

    nc.compile()
    return nc


def _get_nc():
    if "nc" not in _CACHE:
        _CACHE["nc"] = _build()
    return _CACHE["nc"]


def _in_maps(x, Wq, bq, Wk, Wv, bv, Wo, bo):
    qs = LOG2E / 8.0
    maps = []
    for core in range(8):
        b, g = core // 2, core % 2
        cols = slice(g * COLS, (g + 1) * COLS)
        maps.append({
            "xt": np.ascontiguousarray(x[b].T).astype(NP_BF16),
            "wqt": np.ascontiguousarray((Wq[cols] * qs).T).astype(NP_BF16),
            "bq": np.ascontiguousarray((bq[cols] * qs).reshape(CC, 128).T),
            "wkt": np.ascontiguousarray(Wk[cols].T).astype(NP_BF16),
            "wvt": np.ascontiguousarray(Wv[cols].T).astype(NP_BF16),
            "bv": bv[cols].reshape(1, COLS).copy(),
            "wot": np.ascontiguousarray(Wo[:, cols].T).astype(NP_BF16),
            "bo2": np.ascontiguousarray((bo / 2.0).reshape(DCH, 128).T),
        })
    return maps


def kernel(x, Wq, bq, Wk, bk, Wv, bv, Wo, bo, _trace=False, **trace_kwargs):
    x = np.asarray(x, dtype=np.float32)
    Wq = np.asarray(Wq, dtype=np.float32)
    bq = np.asarray(bq, dtype=np.float32)
    Wk = np.asarray(Wk, dtype=np.float32)
    Wv = np.asarray(Wv, dtype=np.float32)
    bv = np.asarray(bv, dtype=np.float32)
    Wo = np.asarray(Wo, dtype=np.float32)
    bo = np.asarray(bo, dtype=np.float32)

    nc = _get_nc()
    maps = _in_maps(x, Wq, bq, Wk, Wv, bv, Wo, bo)
    res = run_bass_kernel_spmd(nc, maps, list(range(8)), trace=_trace, **trace_kwargs)

    outp = np.empty((B, S, D), np.float32)
    for b in range(B):
        t = res.results[2 * b]["out"] + res.results[2 * b + 1]["out"]
        outp[b] = t.T
    if _trace:
        return outp, res
    return outp
